# revision 1
# baseline (speedup 1.0000x reference)
"""Trainium2 Bass kernel for nn_E2ECompressedGEBDModel (SPoS GEBD model).

Full pipeline per window: 2-layer LSTM (seq len 17) -> per-group cosine
self-similarity (4 groups x 17x17) -> 4x (5x5 conv + BN + ReLU) -> global
mean pool.  Data-parallel over the 1024 independent windows: core c gets
SPoS offset c (128 windows).  Window extraction / scatter-back are pure
index gathers done host-side; all FLOPs run on-device.

Self-contained: hardcodes all shapes; does not read ./reference.py etc.
"""

import math
import sys
import types

import numpy as np

K = 8
DIM = 256
GROUP = 4
B, T = 4, 256
NW = T // K              # 32 windows per (batch, offset)
L = 2 * K + 1            # 17 sequence length
NCORES = 8
NSAMP = B * NW           # 128 windows per core
H = DIM
GD = DIM // GROUP        # 64


def _install_ntff_hook():
    """The agent image's antenv lacks axon_hooks; synthesize it so
    run_bass_kernel_spmd(trace=True) can capture NTFF profiles."""
    if "antenv.axon_hooks" in sys.modules:
        return
    import antenv

    hooks_mod = types.ModuleType("antenv.axon_hooks")
    _hook = [None]
    hooks_mod.set_axon_ntff_profile_hook = lambda h: _hook.__setitem__(0, h)
    hooks_mod.get_axon_ntff_profile_hook = lambda: _hook[0]
    sys.modules["antenv.axon_hooks"] = hooks_mod
    antenv.axon_hooks = hooks_mod
    try:
        from trn_agent_boot.trn_boot import _ntff_profile_via_ctypes

        hooks_mod.set_axon_ntff_profile_hook(
            _ntff_profile_via_ctypes("/opt/axon/libaxon_pjrt.so")
        )
    except Exception:
        pass


_LDW_PATCHED = []


def _enable_ldw_opt():
    """walrus dedupes back-to-back LDWEIGHTS of the same stationary operand
    only when this flag is on; our conv loops emit sample-pairs sharing
    weights, so enable it."""
    if _LDW_PATCHED:
        return
    import concourse.bass_utils as _bu

    _orig = _bu.run_command

    def _patched(argv, **kw):
        argv = ["--enable-ldw-opt=true" if a == "--enable-ldw-opt=false" else a
                for a in argv]
        return _orig(argv, **kw)

    _bu.run_command = _patched
    _LDW_PATCHED.append(True)


def build_program(nsamp=NSAMP, stop_after=None):
    """Build + compile the per-core Bass program (SPMD, identical on all
    cores).  nsamp is the number of windows this program handles (small
    values used for CoreSim validation)."""
    import concourse.bass as bass
    import concourse.mybir as mybir
    import concourse.tile as tile
    from concourse import bacc
    from concourse.masks import make_identity

    dt = mybir.dt
    f32, f32r, f16 = dt.float32, dt.float32r, dt.float16
    AF = mybir.ActivationFunctionType
    NS = nsamp
    ts = bass.ts

    import os
    if os.environ.get("KLDWOPT"):
        _enable_ldw_opt()
    nc = bacc.Bacc("TRN2", target_bir_lowering=False, debug=False,
                   num_devices=NCORES)

    # ---- DRAM I/O --------------------------------------------------------
    d_xcatT = nc.dram_tensor("xcatT", [256, L * NS], f16, kind="ExternalInput")
    d_wc = [nc.dram_tensor(f"wc{l}", [512, 1024], f16, kind="ExternalInput")
            for l in range(2)]
    d_bias = [nc.dram_tensor(f"bias{l}", [1, 1024], f16, kind="ExternalInput")
              for l in range(2)]
    d_w0 = nc.dram_tensor("w0", [20, 5 * 2 * 128], f16, kind="ExternalInput")
    d_wconv = [nc.dram_tensor(f"w{l}", [256, 25 * 2 * 128], f16,
                              kind="ExternalInput") for l in (1, 2, 3)]
    d_bn = nc.dram_tensor("bn", [256, 8], f32, kind="ExternalInput")
    d_out = nc.dram_tensor("hout", [256, NS], f32, kind="ExternalOutput")

    with tile.TileContext(nc) as tc:
        wpool = tc.alloc_tile_pool(name="weights", bufs=1)
        work = tc.alloc_tile_pool(name="work", bufs=1)

        # ---- persistent SBUF tensors ------------------------------------
        sb_wc = wpool.tile([128, 2, 4, 1024], f16, tag="wc")
        for l in range(2):
            nc.sync.dma_start(
                out=sb_wc[:, l],
                in_=d_wc[l].ap().rearrange("(a p) n -> p a n", p=128))
        sb_bias = wpool.tile([33, 1024], f16, tag="bias")
        for l in range(2):
            nc.sync.dma_start(out=sb_bias[32 * l:32 * l + 1, :],
                              in_=d_bias[l].ap())
        sb_w0 = wpool.tile([128, 5, 2, 128], f16, tag="w0")
        nc.sync.dma_start(
            out=sb_w0[0:20],
            in_=d_w0.ap().rearrange("p (dx mt m) -> p dx mt m", dx=5, mt=2))
        sb_bn = wpool.tile([128, 2, 8], f32, tag="bn")
        nc.sync.dma_start(out=sb_bn,
                          in_=d_bn.ap().rearrange("(a p) n -> p a n", p=128))
        sb_ones = wpool.tile([33, 128], f16, tag="ones")
        nc.vector.memset(sb_ones[0:1], 1.0)
        nc.vector.memset(sb_ones[32:33], 1.0)
        sb_ident = wpool.tile([128, 128], f32, tag="ident")
        make_identity(nc, sb_ident)

        # LSTM state
        sb_h = work.tile([128, 2, 256], f32, tag="h")        # [s, layer, hid]
        sb_c = work.tile([128, 2, 256], f32, tag="c")
        nc.vector.memset(sb_h, 0.0)
        nc.vector.memset(sb_c, 0.0)
        h0T = work.tile([128, 2, NS, L], f16, tag="h0T")     # [hid_p, kt, s, t]
        hhT = work.tile([128, 2, 2, NS], f16, tag="hhT")     # [hid_p, t%2, kt, s]
        hnT = work.tile([128, 2, NS, L], f16, tag="hnT")     # normalized h1^T
        sb_sig = work.tile([128, 2, 768], f32, tag="sig")
        sb_gg = work.tile([128, 2, 256], f32, tag="gg")
        sb_ig = work.tile([128, 2, 256], f32, tag="ig")
        sb_tc = work.tile([128, 2, 256], f32, tag="tc")
        sb_hn = work.tile([128, 256], f32, tag="hn")
        sb_sq = work.tile([128, 256], f32, tag="sq")
        sb_ss = work.tile([128, 4], f32, tag="ss")
        sb_sr = work.tile([128, 4], f32, tag="sr")
        sb_rn = work.tile([128, 4], f32, tag="rn")
        sb_eps = work.tile([128, 1], f32, tag="eps")
        nc.vector.memset(sb_eps, 1e-8)

        # conv stage persistent buffers
        zp = work.tile([128, 2, 128], f16, tag="zp")
        nc.vector.memset(zp, 0.0)
        s1 = [work.tile([128, 17], f16, tag=f"s1_{i}", name=f"s1_{i}")
              for i in range(2)]
        sim_pad = work.tile([128, 2, 441], f16, tag="sim_pad")   # [g(4), pp, 21*21]
        im2col = work.tile([128, 2, 357], f16, tag="im2col")     # [(g,dy)(20), pp, 17*21]
        act_pad = [work.tile([128, 2, 2, 441], f16, tag=f"act{i}",
                              name=f"act{i}")
                   for i in range(3)]                            # [p, pp, kt, 21*21]
        scratch = work.tile([128, 2, 289], f32, tag="scratch")
        hout_sb = work.tile([128, 2, NS], f32, tag="hout_sb")
        nc.vector.memset(hout_sb, 0.0)
        nc.vector.memset(sim_pad, 0.0)
        for a in act_pad:
            nc.vector.memset(a, 0.0)

        def interior(ap441):
            """(128, 441) padded-image AP -> (128, 17, 17) interior view."""
            return ap441.rearrange("p (r c) -> p r c", c=21)[:, 2:19, 2:19]

        # ================= LSTM (2 layers, interleaved) ===================
        xcpool = tc.alloc_tile_pool(name="xcpool", bufs=1)
        sb_xcatT = xcpool.tile([128, 2, L * NS], f16, tag="xcatT")
        nc.sync.dma_start(out=sb_xcatT,
                          in_=d_xcatT.ap().rearrange("(a p) n -> p a n", p=128))
        psz = tc.alloc_tile_pool(name="psz", bufs=2, space="PSUM")
        pst = tc.alloc_tile_pool(name="pst", bufs=4, space="PSUM")

        def lstm_step(layer, t):
            ps = psz.tile([128, 1024], f32, tag="z")
            # bias (k=1 matmul with ones row) — also opens the accum group
            for nh in range(2):
                nc.tensor.matmul(
                    ps[0:NS, ts(nh, 512)],
                    sb_ones[32 * layer:32 * layer + 1, 0:NS],
                    sb_bias[32 * layer:32 * layer + 1, ts(nh, 512)],
                    start=True, stop=False)
            # input contribution
            for kt in range(2):
                if layer == 0:
                    lhsT = sb_xcatT[:, kt, t * NS:(t + 1) * NS]
                else:
                    lhsT = h0T[:, kt, :, t]
                for nh in range(2):
                    nc.tensor.matmul(
                        ps[0:NS, ts(nh, 512)], lhsT,
                        sb_wc[:, layer, kt, ts(nh, 512)],
                        start=False, stop=(t == 0 and kt == 1))
            # recurrent contribution (h_{-1} = 0 -> skip at t=0)
            if t > 0:
                for kt in range(2):
                    if layer == 0:
                        lhsT = h0T[:, kt, :, t - 1]
                    else:
                        lhsT = hhT[:, (t - 1) % 2, kt, :]
                    for nh in range(2):
                        nc.tensor.matmul(
                            ps[0:NS, ts(nh, 512)], lhsT,
                            sb_wc[:, layer, 2 + kt, ts(nh, 512)],
                            start=False, stop=(kt == 1))
            # gates: layout [i(0:256) f(256:512) o(512:768) | g(768:1024)]
            sig = sb_sig[0:NS, layer]
            nc.scalar.activation(sig, ps[0:NS, 0:768], AF.Sigmoid)
            nc.scalar.activation(sb_gg[0:NS, layer], ps[0:NS, 768:1024], AF.Tanh)
            c_ = sb_c[0:NS, layer]
            h_ = sb_h[0:NS, layer]
            nc.vector.tensor_mul(sb_ig[0:NS, layer], sig[:, 0:256],
                                 sb_gg[0:NS, layer])
            if t > 0:
                nc.vector.tensor_mul(c_, sig[:, 256:512], c_)
                nc.vector.tensor_add(c_, c_, sb_ig[0:NS, layer])
            else:
                nc.vector.tensor_copy(c_, sb_ig[0:NS, layer])
            nc.scalar.activation(sb_tc[0:NS, layer], c_, AF.Tanh)
            nc.vector.tensor_mul(h_, sig[:, 512:768], sb_tc[0:NS, layer])
            # transpose h_t (raw) for recurrence / next-layer input
            for kt in range(2):
                pt = pst.tile([128, NS], f32, tag="tr")
                nc.tensor.transpose(pt, h_[:, ts(kt, 128)],
                                    sb_ident[0:NS, 0:NS])
                dest = h0T[:, kt, :, t] if layer == 0 else hhT[:, t % 2, kt, :]
                nc.vector.tensor_copy(dest, pt)
            if layer == 1:
                # normalize per similarity group, transpose into hnT
                nc.vector.tensor_mul(sb_sq[0:NS], h_, h_)
                nc.vector.reduce_sum(
                    sb_ss[0:NS],
                    sb_sq[0:NS].rearrange("p (g d) -> p g d", g=4),
                    axis=mybir.AxisListType.X)
                nc.scalar.activation(sb_sr[0:NS], sb_ss[0:NS], AF.Sqrt,
                                     bias=sb_eps[0:NS])
                nc.vector.reciprocal(sb_rn[0:NS], sb_sr[0:NS])
                for g in range(4):
                    nc.vector.tensor_scalar_mul(
                        sb_hn[0:NS, ts(g, GD)], h_[:, ts(g, GD)],
                        sb_rn[0:NS, g:g + 1])
                for kt in range(2):
                    pt = pst.tile([128, NS], f32, tag="tr")
                    nc.tensor.transpose(pt, sb_hn[0:NS, ts(kt, 128)],
                                        sb_ident[0:NS, 0:NS])
                    nc.vector.tensor_copy(hnT[:, kt, :, t], pt)

        # software-pipeline the two layers: L1 runs one step behind L0
        lstm_step(0, 0)
        lstm_step(0, 1)
        for t in range(2, L + 2):
            if t < L:
                lstm_step(0, t)
            lstm_step(1, t - 2)
        pst.release()
        psz.release()
        xcpool.release()

        # ================= similarity + convs, per sample =================
        cwpool = tc.alloc_tile_pool(name="cwpool", bufs=1)
        sb_wconv = cwpool.tile([128, 3, 2, 25, 2, 128], f16, tag="wconv")
        for i in range(3):
            nc.sync.dma_start(
                out=sb_wconv[:, i],
                in_=d_wconv[i].ap().rearrange("(a p) n -> p a n", p=128))
        psim = tc.alloc_tile_pool(name="psim", bufs=2, space="PSUM")
        pcv = tc.alloc_tile_pool(name="pcv", bufs=1, space="PSUM")
        pc_t = [pcv.tile([128, 2, 512], f32, tag=f"pc{i}", name=f"pc{i}")
                for i in range(2)]

        def sim_conv0(s, mms=True):
            pp = s % 2
            # stationary zp: column block [32g : 32g+17] holds group g's
            # normalized vectors (rows = hidden slice of that group, zeros
            # elsewhere) -> one matmul accumulation group computes all 4
            # group-dot blocks into psum partitions [32g:32g+17].
            for g in range(4):
                kt, ko = g // 2, (g % 2) * GD
                nc.vector.tensor_copy(
                    zp[ko:ko + GD, kt, 32 * g:32 * g + 17],
                    hnT[ko:ko + GD, kt, s, :])
            ps = psim.tile([128, 17], f32, tag="psim")
            for kt in range(2):
                nc.tensor.matmul(ps, zp[:, kt, :], hnT[:, kt, s, :],
                                 start=(kt == 0), stop=(kt == 1))
            nc.vector.tensor_copy(s1[pp], ps)
            # regroup [32g+i, j] -> image [g, i(row), j(col)], zero border
            for g in range(4):
                dst = sim_pad[g:g + 1, pp].rearrange(
                    "p (r c) -> p r c", c=21)[:, 2:19, 2:19]
                nc.sync.dma_start(out=dst, in_=s1[pp][32 * g:32 * g + 17, :])
            # im2col over (g, dy): row (g,dy) = sim_pad[g, dy:dy+17, :]
            sp = sim_pad[0:4, pp]
            src = bass.AP(tensor=sp.tensor, offset=sp.offset,
                          ap=[sp.ap[0], [21, 5], [1, 357]])
            nc.sync.dma_start(out=im2col[0:20, pp], in_=src)

        def conv0_pair(pair):
            for s in pair:
                pp = s % 2
                im = im2col[0:20, pp].rearrange("p (r c) -> p r c", c=21)
                for mt in range(2):
                    for dx in range(5):
                        nc.tensor.matmul(pc_t[pp][:, mt, 0:289],
                                         sb_w0[0:20, dx, mt, :],
                                         im[:, :, dx:dx + 17],
                                         start=(dx == 0), stop=(dx == 4))
                    nc.scalar.activation(
                        interior(act_pad[0][:, pp, mt]),
                        pc_t[pp][:, mt, 0:289].rearrange(
                            "p (r c) -> p r c", c=17),
                        AF.Relu, scale=sb_bn[:, mt, 0:1],
                        bias=sb_bn[:, mt, 1:2])

        def conv_pair(pair, lyr):
            """lyr in 1..3: act_pad[lyr-1] -> act_pad[lyr] (or pooled out)."""
            src = act_pad[lyr - 1]
            for s in pair:
                pp = s % 2
                pc = pc_t[pp]
                for mt in range(2):
                    first = True
                    for kt in range(2):
                        rr = src[:, pp, kt].rearrange("p (r c) -> p r c", c=21)
                        for dy in range(5):
                            for dx in range(5):
                                tau = dy * 5 + dx
                                nc.tensor.matmul(
                                    pc[:, mt, 0:289],
                                    sb_wconv[:, lyr - 1, kt, tau, mt, :],
                                    rr[:, dy:dy + 17, dx:dx + 17],
                                    start=first,
                                    stop=(kt == 1 and tau == 24))
                                first = False
                    if lyr < 3:
                        nc.scalar.activation(
                            interior(act_pad[lyr][:, pp, mt]),
                            pc[:, mt, 0:289].rearrange("p (r c) -> p r c", c=17),
                            AF.Relu, scale=sb_bn[:, mt, 2 * lyr:2 * lyr + 1],
                            bias=sb_bn[:, mt, 2 * lyr + 1:2 * lyr + 2])
                    else:
                        nc.scalar.activation(
                            scratch[:, mt], pc[:, mt, 0:289], AF.Relu,
                            scale=sb_bn[:, mt, 6:7], bias=sb_bn[:, mt, 7:8],
                            accum_out=hout_sb[:, mt, s:s + 1])

        # pair-pipelined emission
        if stop_after != "lstm":
            last = {"sim": 0, "conv0": 0, "conv1": 1, "conv2": 2}.get(
                stop_after, 3)
            for p in range(0, nsamp, 2):
                pair = [p] if p + 1 >= nsamp else [p, p + 1]
                for s in pair:
                    sim_conv0(s, mms=(stop_after != "sim"))
                if stop_after != "sim":
                    conv0_pair(pair)
                for lyr in (1, 2, 3):
                    if lyr > last:
                        break
                    conv_pair(pair, lyr)

        nc.sync.dma_start(
            out=d_out.ap().rearrange("(a p) n -> p a n", p=128),
            in_=hout_sb[:, :, 0:NS])

        pcv.release()
        psim.release()
        cwpool.release()
        work.release()
        wpool.release()

    nc.compile()
    return nc


# ======================= host-side preparation ===========================

def make_xcat(x):
    """Window extraction, identical to the reference (pL == T case)."""
    x = np.asarray(x, np.float32)
    lefts, rights, mids = [], [], []
    for offset in range(K):
        s = K - offset
        left = np.concatenate(
            [np.repeat(x[:, :, :1], s, axis=2), x[:, :, :-s]], axis=2)
        r = offset + 1
        right = np.concatenate(
            [x[:, :, r:], np.repeat(x[:, :, -1:], r, axis=2)], axis=2)
        lefts.append(left.reshape(B, DIM, NW, K).transpose(0, 2, 3, 1)
                     .reshape(B * NW, K, DIM))
        rights.append(right.reshape(B, DIM, NW, K).transpose(0, 2, 3, 1)
                      .reshape(B * NW, K, DIM))
        mids.append(x[:, :, offset::K].transpose(0, 2, 1)
                    .reshape(B * NW, 1, DIM))
    left_seq = np.concatenate(lefts, axis=0)
    right_seq = np.concatenate(rights, axis=0)
    mid_seq = np.concatenate(mids, axis=0)
    return np.concatenate([left_seq, mid_seq, right_seq], axis=1)  # (1024,17,256)


def prep_weights(inp):
    """Host-side reorder of parameters into the device layouts."""
    g = {}
    perm = np.concatenate([np.arange(0, 256), np.arange(256, 512),
                           np.arange(768, 1024), np.arange(512, 768)])
    for l in range(2):
        wih = np.asarray(inp[f"w_ih{l}"], np.float32)[perm]
        whh = np.asarray(inp[f"w_hh{l}"], np.float32)[perm]
        g[f"wc{l}"] = np.ascontiguousarray(
            np.vstack([wih.T, whh.T]), dtype=np.float16)        # (512,1024)
        g[f"bias{l}"] = np.ascontiguousarray(
            (np.asarray(inp[f"b_ih{l}"], np.float32)
             + np.asarray(inp[f"b_hh{l}"], np.float32))[perm][None, :],
            dtype=np.float16)
    # conv0 layout: [(g,dy), (dx, mt, m)]
    w0 = np.asarray(inp["conv0_w"], np.float32)                 # (256,4,5,5)
    t = w0.transpose(1, 2, 3, 0)            # (g, dy, dx, cout)
    t = t.reshape(4, 5, 5, 2, 128)          # (g, dy, dx, mt, m)
    t = t.transpose(0, 1, 2, 3, 4).reshape(20, 5, 2, 128)  # rows (g,dy)
    g["w0"] = np.ascontiguousarray(t.reshape(20, 5 * 2 * 128),
                                   dtype=np.float16)
    for i, name in enumerate(("conv1_w", "conv2_w", "conv3_w")):
        w = np.asarray(inp[name], np.float32)                   # (256,256,5,5)
        t = w.transpose(1, 2, 3, 0)         # (cin, dy, dx, cout)
        t = t.reshape(2, 128, 25, 2, 128)   # (kt, p, tau, mt, m)
        g[f"w{i + 1}"] = np.ascontiguousarray(
            t.reshape(256, 25 * 2 * 128), dtype=np.float16)
    bn = np.zeros((256, 8), np.float32)
    for i in range(4):
        s = np.asarray(inp[f"bn{i}_s"], np.float32)
        b = np.asarray(inp[f"bn{i}_b"], np.float32)
        if i == 3:
            s = s / 289.0
            b = b / 289.0
        bn[:, 2 * i] = s
        bn[:, 2 * i + 1] = b
    g["bn"] = bn
    return g


_CACHE = {}


def kernel(**inputs):
    _install_ntff_hook()
    from concourse.bass_utils import run_bass_kernel_spmd

    if "nc" not in _CACHE:
        _CACHE["nc"] = build_program(NSAMP)
    nc = _CACHE["nc"]

    shared = prep_weights(inputs)
    xcat = make_xcat(inputs["x"])           # (1024, 17, 256)
    in_maps = []
    for c in range(NCORES):
        xc = xcat[c * NSAMP:(c + 1) * NSAMP]            # (128, 17, 256)
        xcT = np.ascontiguousarray(
            xc.transpose(2, 1, 0).reshape(256, L * NSAMP), dtype=np.float16)
        m = dict(shared)
        m["xcatT"] = xcT
        in_maps.append(m)

    res = run_bass_kernel_spmd(nc, in_maps, core_ids=list(range(NCORES)))
    out = np.zeros((B, DIM, T), np.float32)
    for c in range(NCORES):
        hc = res.results[c]["hout"].T                   # (128, 256)
        out[:, :, c::K] = hc.reshape(B, NW, DIM).transpose(0, 2, 1)
    return out



# revision 2
# speedup vs baseline: 1.0691x; 1.0691x over previous
"""Trainium2 Bass kernel for nn_E2ECompressedGEBDModel (SPoS GEBD model).

Full pipeline per window: 2-layer LSTM (seq len 17) -> per-group cosine
self-similarity (4 groups x 17x17) -> 4x (5x5 conv + BN + ReLU) -> global
mean pool.  Data-parallel over the 1024 independent windows: core c gets
SPoS offset c (128 windows).  Window extraction / scatter-back are pure
index gathers done host-side; all FLOPs run on-device.

Self-contained: hardcodes all shapes; does not read ./reference.py etc.
"""

import math
import sys
import types

import numpy as np

K = 8
DIM = 256
GROUP = 4
B, T = 4, 256
NW = T // K              # 32 windows per (batch, offset)
L = 2 * K + 1            # 17 sequence length
NCORES = 8
NSAMP = B * NW           # 128 windows per core
H = DIM
GD = DIM // GROUP        # 64

# fp8 e4m3 scaling for conv1-3 (folded into weights + bn tables host-side)
SW = 256.0               # weight scale, conv1-3
SA = (16.0, 16.0, 16.0)  # activation scale for act0/act1/act2 (conv1-3 inputs)


def _install_ntff_hook():
    """The agent image's antenv lacks axon_hooks; synthesize it so
    run_bass_kernel_spmd(trace=True) can capture NTFF profiles."""
    if "antenv.axon_hooks" in sys.modules:
        return
    import antenv

    hooks_mod = types.ModuleType("antenv.axon_hooks")
    _hook = [None]
    hooks_mod.set_axon_ntff_profile_hook = lambda h: _hook.__setitem__(0, h)
    hooks_mod.get_axon_ntff_profile_hook = lambda: _hook[0]
    sys.modules["antenv.axon_hooks"] = hooks_mod
    antenv.axon_hooks = hooks_mod
    try:
        from trn_agent_boot.trn_boot import _ntff_profile_via_ctypes

        hooks_mod.set_axon_ntff_profile_hook(
            _ntff_profile_via_ctypes("/opt/axon/libaxon_pjrt.so")
        )
    except Exception:
        pass


_LDW_PATCHED = []


def _enable_ldw_opt():
    """walrus dedupes back-to-back LDWEIGHTS of the same stationary operand
    only when this flag is on; our conv loops emit sample-pairs sharing
    weights, so enable it."""
    if _LDW_PATCHED:
        return
    import concourse.bass_utils as _bu

    _orig = _bu.run_command

    def _patched(argv, **kw):
        argv = ["--enable-ldw-opt=true" if a == "--enable-ldw-opt=false" else a
                for a in argv]
        return _orig(argv, **kw)

    _bu.run_command = _patched
    _LDW_PATCHED.append(True)


def build_program(nsamp=NSAMP, stop_after=None):
    """Build + compile the per-core Bass program (SPMD, identical on all
    cores).  nsamp is the number of windows this program handles (small
    values used for CoreSim validation)."""
    import concourse.bass as bass
    import concourse.mybir as mybir
    import concourse.tile as tile
    from concourse import bacc
    from concourse.masks import make_identity

    dt = mybir.dt
    f32, f32r, f16 = dt.float32, dt.float32r, dt.float16
    f8 = dt.float8e4
    AF = mybir.ActivationFunctionType
    NS = nsamp
    ts = bass.ts

    import os
    if os.environ.get("KLDWOPT"):
        _enable_ldw_opt()
    nc = bacc.Bacc("TRN2", target_bir_lowering=False, debug=False,
                   num_devices=NCORES)

    # ---- DRAM I/O --------------------------------------------------------
    d_xcatT = nc.dram_tensor("xcatT", [256, L * NS], f16, kind="ExternalInput")
    d_wc = [nc.dram_tensor(f"wc{l}", [512, 1024], f16, kind="ExternalInput")
            for l in range(2)]
    d_bias = [nc.dram_tensor(f"bias{l}", [1, 1024], f16, kind="ExternalInput")
              for l in range(2)]
    d_w0 = nc.dram_tensor("w0", [20, 5 * 2 * 128], f16, kind="ExternalInput")
    d_wconv = [nc.dram_tensor(f"w{l}", [128, 25 * 2 * 2 * 128], f8,
                              kind="ExternalInput") for l in (1, 2, 3)]
    d_bn = nc.dram_tensor("bn", [256, 8], f32, kind="ExternalInput")
    d_corr = nc.dram_tensor("corr", [128, 2 * 3 * 2 * 128], f32,
                            kind="ExternalInput")
    d_out = nc.dram_tensor("hout", [256, NS], f32, kind="ExternalOutput")

    with tile.TileContext(nc) as tc:
        wpool = tc.alloc_tile_pool(name="weights", bufs=1)
        work = tc.alloc_tile_pool(name="work", bufs=1)

        # ---- persistent SBUF tensors ------------------------------------
        sb_wc = wpool.tile([128, 2, 4, 1024], f16, tag="wc")
        for l in range(2):
            nc.sync.dma_start(
                out=sb_wc[:, l],
                in_=d_wc[l].ap().rearrange("(a p) n -> p a n", p=128))
        sb_bias = wpool.tile([33, 1024], f16, tag="bias")
        for l in range(2):
            nc.sync.dma_start(out=sb_bias[32 * l:32 * l + 1, :],
                              in_=d_bias[l].ap())
        sb_w0 = wpool.tile([128, 5, 2, 128], f16, tag="w0")
        nc.sync.dma_start(
            out=sb_w0[0:20],
            in_=d_w0.ap().rearrange("p (dx mt m) -> p dx mt m", dx=5, mt=2))
        sb_bn = wpool.tile([128, 2, 8], f32, tag="bn")
        nc.sync.dma_start(out=sb_bn,
                          in_=d_bn.ap().rearrange("(a p) n -> p a n", p=128))
        sb_ones = wpool.tile([33, 128], f16, tag="ones")
        nc.vector.memset(sb_ones[0:1], 1.0)
        nc.vector.memset(sb_ones[32:33], 1.0)
        sb_ident = wpool.tile([128, 128], f32, tag="ident")
        make_identity(nc, sb_ident)

        # LSTM state
        sb_h = work.tile([128, 2, 256], f32, tag="h")        # [s, layer, hid]
        sb_c = work.tile([128, 2, 256], f32, tag="c")
        nc.vector.memset(sb_h, 0.0)
        nc.vector.memset(sb_c, 0.0)
        h0T = work.tile([128, 2, NS, L], f16, tag="h0T")     # [hid_p, kt, s, t]
        hhT = work.tile([128, 2, 2, NS], f16, tag="hhT")     # [hid_p, t%2, kt, s]
        hnT = work.tile([128, 2, NS, L], f16, tag="hnT")     # normalized h1^T
        sb_sig = work.tile([128, 2, 768], f32, tag="sig")
        sb_gg = work.tile([128, 2, 256], f32, tag="gg")
        sb_ig = work.tile([128, 2, 256], f32, tag="ig")
        sb_tc = work.tile([128, 2, 256], f32, tag="tc")
        sb_hn = work.tile([128, 256], f32, tag="hn")
        sb_sq = work.tile([128, 256], f32, tag="sq")
        sb_ss = work.tile([128, 4], f32, tag="ss")
        sb_sr = work.tile([128, 4], f32, tag="sr")
        sb_rn = work.tile([128, 4], f32, tag="rn")
        sb_eps = work.tile([128, 1], f32, tag="eps")
        nc.vector.memset(sb_eps, 1e-8)

        # conv stage persistent buffers
        zp = work.tile([128, 2, 128], f16, tag="zp")
        nc.vector.memset(zp, 0.0)
        s1 = [work.tile([128, 17], f16, tag=f"s1_{i}", name=f"s1_{i}")
              for i in range(2)]
        sim_pad = work.tile([128, 2, 441], f16, tag="sim_pad")   # [g(4), pp, 21*21]
        im2col = work.tile([128, 2, 357], f16, tag="im2col")     # [(g,dy)(20), pp, 17*21]
        act_pad = [work.tile([128, 2, 2, 448], f8, tag=f"act{i}",
                              name=f"act{i}")
                   for i in range(3)]                            # [p, pp, kt, 21*21+7]
        scratch = work.tile([128, 2, 289], f32, tag="scratch")
        hout_sb = work.tile([128, 2, NS], f32, tag="hout_sb")
        # fp8 weight-error correction state: per-window channel-mean acts
        # (accum_out of the producing layer) and the per-window bias vector.
        abar = work.tile([128, 2, 3, 2, 1], f32, tag="abar")   # [p, pp, act, kt]
        bcorr = work.tile([128, 2, 3, 2, 1], f32, tag="bcorr")  # [p, pp, lyr-1, mt]
        nc.vector.memset(hout_sb, 0.0)
        nc.vector.memset(sim_pad, 0.0)
        for a in act_pad:
            nc.vector.memset(a, 0.0)

        def interior(ap441):
            """(128, 441+) padded-image AP -> (128, 17, 17) interior view."""
            return ap441[:, 0:441].rearrange(
                "p (r c) -> p r c", c=21)[:, 2:19, 2:19]

        # ================= LSTM (2 layers, interleaved) ===================
        xcpool = tc.alloc_tile_pool(name="xcpool", bufs=1)
        sb_xcatT = xcpool.tile([128, 2, L * NS], f16, tag="xcatT")
        nc.sync.dma_start(out=sb_xcatT,
                          in_=d_xcatT.ap().rearrange("(a p) n -> p a n", p=128))
        psz = tc.alloc_tile_pool(name="psz", bufs=2, space="PSUM")
        pst = tc.alloc_tile_pool(name="pst", bufs=4, space="PSUM")

        def lstm_step(layer, t):
            ps = psz.tile([128, 1024], f32, tag="z")
            # bias (k=1 matmul with ones row) — also opens the accum group
            for nh in range(2):
                nc.tensor.matmul(
                    ps[0:NS, ts(nh, 512)],
                    sb_ones[32 * layer:32 * layer + 1, 0:NS],
                    sb_bias[32 * layer:32 * layer + 1, ts(nh, 512)],
                    start=True, stop=False)
            # input contribution
            for kt in range(2):
                if layer == 0:
                    lhsT = sb_xcatT[:, kt, t * NS:(t + 1) * NS]
                else:
                    lhsT = h0T[:, kt, :, t]
                for nh in range(2):
                    nc.tensor.matmul(
                        ps[0:NS, ts(nh, 512)], lhsT,
                        sb_wc[:, layer, kt, ts(nh, 512)],
                        start=False, stop=(t == 0 and kt == 1))
            # recurrent contribution (h_{-1} = 0 -> skip at t=0)
            if t > 0:
                for kt in range(2):
                    if layer == 0:
                        lhsT = h0T[:, kt, :, t - 1]
                    else:
                        lhsT = hhT[:, (t - 1) % 2, kt, :]
                    for nh in range(2):
                        nc.tensor.matmul(
                            ps[0:NS, ts(nh, 512)], lhsT,
                            sb_wc[:, layer, 2 + kt, ts(nh, 512)],
                            start=False, stop=(kt == 1))
            # gates: layout [i(0:256) f(256:512) o(512:768) | g(768:1024)]
            sig = sb_sig[0:NS, layer]
            nc.scalar.activation(sig, ps[0:NS, 0:768], AF.Sigmoid)
            nc.scalar.activation(sb_gg[0:NS, layer], ps[0:NS, 768:1024], AF.Tanh)
            c_ = sb_c[0:NS, layer]
            h_ = sb_h[0:NS, layer]
            nc.vector.tensor_mul(sb_ig[0:NS, layer], sig[:, 0:256],
                                 sb_gg[0:NS, layer])
            if t > 0:
                nc.vector.tensor_mul(c_, sig[:, 256:512], c_)
                nc.vector.tensor_add(c_, c_, sb_ig[0:NS, layer])
            else:
                nc.vector.tensor_copy(c_, sb_ig[0:NS, layer])
            nc.scalar.activation(sb_tc[0:NS, layer], c_, AF.Tanh)
            nc.vector.tensor_mul(h_, sig[:, 512:768], sb_tc[0:NS, layer])
            # transpose h_t (raw) for recurrence / next-layer input
            for kt in range(2):
                pt = pst.tile([128, NS], f32, tag="tr")
                nc.tensor.transpose(pt, h_[:, ts(kt, 128)],
                                    sb_ident[0:NS, 0:NS])
                dest = h0T[:, kt, :, t] if layer == 0 else hhT[:, t % 2, kt, :]
                nc.vector.tensor_copy(dest, pt)
            if layer == 1:
                # normalize per similarity group, transpose into hnT
                nc.vector.tensor_mul(sb_sq[0:NS], h_, h_)
                nc.vector.reduce_sum(
                    sb_ss[0:NS],
                    sb_sq[0:NS].rearrange("p (g d) -> p g d", g=4),
                    axis=mybir.AxisListType.X)
                nc.scalar.activation(sb_sr[0:NS], sb_ss[0:NS], AF.Sqrt,
                                     bias=sb_eps[0:NS])
                nc.vector.reciprocal(sb_rn[0:NS], sb_sr[0:NS])
                for g in range(4):
                    nc.vector.tensor_scalar_mul(
                        sb_hn[0:NS, ts(g, GD)], h_[:, ts(g, GD)],
                        sb_rn[0:NS, g:g + 1])
                for kt in range(2):
                    pt = pst.tile([128, NS], f32, tag="tr")
                    nc.tensor.transpose(pt, sb_hn[0:NS, ts(kt, 128)],
                                        sb_ident[0:NS, 0:NS])
                    nc.vector.tensor_copy(hnT[:, kt, :, t], pt)

        # software-pipeline the two layers: L1 runs one step behind L0
        lstm_step(0, 0)
        lstm_step(0, 1)
        for t in range(2, L + 2):
            if t < L:
                lstm_step(0, t)
            lstm_step(1, t - 2)
        pst.release()
        psz.release()
        xcpool.release()

        # ================= similarity + convs, per sample =================
        cwpool = tc.alloc_tile_pool(name="cwpool", bufs=1)
        sb_wconv = cwpool.tile([128, 3, 25, 2, 2, 128], f8, tag="wconv")
        for i in range(3):
            nc.sync.dma_start(
                out=sb_wconv[:, i],
                in_=d_wconv[i].ap().rearrange(
                    "p (t m k f) -> p t m k f", t=25, m=2, k=2))
        sb_C = cwpool.tile([128, 2, 3, 2, 128], f32, tag="corrC")
        nc.sync.dma_start(
            out=sb_C,
            in_=d_corr.ap().rearrange("p (k l m f) -> p k l m f", k=2, l=3, m=2))
        psim = tc.alloc_tile_pool(name="psim", bufs=2, space="PSUM")
        pcv = tc.alloc_tile_pool(name="pcv", bufs=1, space="PSUM")
        pc_t = [pcv.tile([128, 2, 512], f32, tag=f"pc{i}", name=f"pc{i}")
                for i in range(2)]
        pcorr = tc.alloc_tile_pool(name="pcorr", bufs=1, space="PSUM")

        def sim_conv0(s, mms=True):
            pp = s % 2
            # stationary zp: column block [32g : 32g+17] holds group g's
            # normalized vectors (rows = hidden slice of that group, zeros
            # elsewhere) -> one matmul accumulation group computes all 4
            # group-dot blocks into psum partitions [32g:32g+17].
            for g in range(4):
                kt, ko = g // 2, (g % 2) * GD
                nc.vector.tensor_copy(
                    zp[ko:ko + GD, kt, 32 * g:32 * g + 17],
                    hnT[ko:ko + GD, kt, s, :])
            ps = psim.tile([128, 17], f32, tag="psim")
            for kt in range(2):
                nc.tensor.matmul(ps, zp[:, kt, :], hnT[:, kt, s, :],
                                 start=(kt == 0), stop=(kt == 1))
            nc.vector.tensor_copy(s1[pp], ps)
            # regroup [32g+i, j] -> image [g, i(row), j(col)], zero border
            for g in range(4):
                dst = sim_pad[g:g + 1, pp].rearrange(
                    "p (r c) -> p r c", c=21)[:, 2:19, 2:19]
                nc.sync.dma_start(out=dst, in_=s1[pp][32 * g:32 * g + 17, :])
            # im2col over (g, dy): row (g,dy) = sim_pad[g, dy:dy+17, :]
            sp = sim_pad[0:4, pp]
            src = bass.AP(tensor=sp.tensor, offset=sp.offset,
                          ap=[sp.ap[0], [21, 5], [1, 357]])
            nc.sync.dma_start(out=im2col[0:20, pp], in_=src)

        def conv0_pair(pair):
            for s in pair:
                pp = s % 2
                im = im2col[0:20, pp].rearrange("p (r c) -> p r c", c=21)
                for mt in range(2):
                    for dx in range(5):
                        nc.tensor.matmul(pc_t[pp][:, mt, 0:289],
                                         sb_w0[0:20, dx, mt, :],
                                         im[:, :, dx:dx + 17],
                                         start=(dx == 0), stop=(dx == 4))
                    nc.scalar.activation(
                        interior(act_pad[0][:, pp, mt]),
                        pc_t[pp][:, mt, 0:289].rearrange(
                            "p (r c) -> p r c", c=17),
                        AF.Relu, scale=sb_bn[:, mt, 0:1],
                        bias=sb_bn[:, mt, 1:2],
                        accum_out=abar[:, pp, 0, mt])

        def conv_pair(pair, lyr):
            """lyr in 1..3: act_pad[lyr-1] -> act_pad[lyr] (or pooled out).
            fp8 DoubleRow: one matmul per tau covers both cin halves."""
            src = act_pad[lyr - 1]
            for s in pair:
                pp = s % 2
                pc = pc_t[pp]
                rr = src[:, pp, :, 0:441].rearrange(
                    "p k (r c) -> p k r c", c=21)
                for mt in range(2):
                    # per-window fp8 weight-error correction:
                    # pcorr = C^T @ abar  (per-cout scalar), folded into bias
                    pco = pcorr.tile([128, 1], f32, tag="pco")
                    for kt in range(2):
                        nc.tensor.matmul(
                            pco, sb_C[:, kt, lyr - 1, mt],
                            abar[:, pp, lyr - 1, kt],
                            start=(kt == 0), stop=(kt == 1))
                    nc.vector.tensor_add(
                        bcorr[:, pp, lyr - 1, mt],
                        sb_bn[:, mt, 2 * lyr + 1:2 * lyr + 2], pco)
                    for dy in range(5):
                        for dx in range(5):
                            tau = dy * 5 + dx
                            nc.tensor.matmul(
                                pc[:, mt, 0:289],
                                sb_wconv[:, lyr - 1, tau, mt],
                                rr[:, :, dy:dy + 17, dx:dx + 17],
                                start=(tau == 0), stop=(tau == 24),
                                perf_mode=mybir.MatmulPerfMode.DoubleRow)
                    if lyr < 3:
                        nc.scalar.activation(
                            interior(act_pad[lyr][:, pp, mt]),
                            pc[:, mt, 0:289].rearrange("p (r c) -> p r c", c=17),
                            AF.Relu, scale=sb_bn[:, mt, 2 * lyr:2 * lyr + 1],
                            bias=bcorr[:, pp, lyr - 1, mt],
                            accum_out=abar[:, pp, lyr, mt])
                    else:
                        nc.scalar.activation(
                            scratch[:, mt], pc[:, mt, 0:289], AF.Relu,
                            scale=sb_bn[:, mt, 6:7],
                            bias=bcorr[:, pp, lyr - 1, mt],
                            accum_out=hout_sb[:, mt, s:s + 1])

        # pair-pipelined emission
        if stop_after != "lstm":
            last = {"sim": 0, "conv0": 0, "conv1": 1, "conv2": 2}.get(
                stop_after, 3)
            for p in range(0, nsamp, 2):
                pair = [p] if p + 1 >= nsamp else [p, p + 1]
                for s in pair:
                    sim_conv0(s, mms=(stop_after != "sim"))
                if stop_after != "sim":
                    conv0_pair(pair)
                for lyr in (1, 2, 3):
                    if lyr > last:
                        break
                    conv_pair(pair, lyr)

        nc.sync.dma_start(
            out=d_out.ap().rearrange("(a p) n -> p a n", p=128),
            in_=hout_sb[:, :, 0:NS])

        pcorr.release()
        pcv.release()
        psim.release()
        cwpool.release()
        work.release()
        wpool.release()

    nc.compile()
    return nc


# ======================= host-side preparation ===========================

def make_xcat(x):
    """Window extraction, identical to the reference (pL == T case)."""
    x = np.asarray(x, np.float32)
    lefts, rights, mids = [], [], []
    for offset in range(K):
        s = K - offset
        left = np.concatenate(
            [np.repeat(x[:, :, :1], s, axis=2), x[:, :, :-s]], axis=2)
        r = offset + 1
        right = np.concatenate(
            [x[:, :, r:], np.repeat(x[:, :, -1:], r, axis=2)], axis=2)
        lefts.append(left.reshape(B, DIM, NW, K).transpose(0, 2, 3, 1)
                     .reshape(B * NW, K, DIM))
        rights.append(right.reshape(B, DIM, NW, K).transpose(0, 2, 3, 1)
                      .reshape(B * NW, K, DIM))
        mids.append(x[:, :, offset::K].transpose(0, 2, 1)
                    .reshape(B * NW, 1, DIM))
    left_seq = np.concatenate(lefts, axis=0)
    right_seq = np.concatenate(rights, axis=0)
    mid_seq = np.concatenate(mids, axis=0)
    return np.concatenate([left_seq, mid_seq, right_seq], axis=1)  # (1024,17,256)


def prep_weights(inp):
    """Host-side reorder of parameters into the device layouts."""
    g = {}
    perm = np.concatenate([np.arange(0, 256), np.arange(256, 512),
                           np.arange(768, 1024), np.arange(512, 768)])
    for l in range(2):
        wih = np.asarray(inp[f"w_ih{l}"], np.float32)[perm]
        whh = np.asarray(inp[f"w_hh{l}"], np.float32)[perm]
        g[f"wc{l}"] = np.ascontiguousarray(
            np.vstack([wih.T, whh.T]), dtype=np.float16)        # (512,1024)
        g[f"bias{l}"] = np.ascontiguousarray(
            (np.asarray(inp[f"b_ih{l}"], np.float32)
             + np.asarray(inp[f"b_hh{l}"], np.float32))[perm][None, :],
            dtype=np.float16)
    # conv0 layout: [(g,dy), (dx, mt, m)]
    w0 = np.asarray(inp["conv0_w"], np.float32)                 # (256,4,5,5)
    t = w0.transpose(1, 2, 3, 0)            # (g, dy, dx, cout)
    t = t.reshape(4, 5, 5, 2, 128)          # (g, dy, dx, mt, m)
    t = t.transpose(0, 1, 2, 3, 4).reshape(20, 5, 2, 128)  # rows (g,dy)
    g["w0"] = np.ascontiguousarray(t.reshape(20, 5 * 2 * 128),
                                   dtype=np.float16)
    import ml_dtypes

    # conv1-3: fp8 e4m3 weights (scaled by SW), layout [p, tau, mt, kt, m];
    # plus per-layer mean-act correction matrices C (see build_program).
    beta = np.zeros((5, 5), np.float32)
    for dy in range(5):
        for dx in range(5):
            beta[dy, dx] = (17 - abs(dy - 2)) * (17 - abs(dx - 2)) / 289.0
    # folded activation scales: scale'_l applied to psum by the activation
    sa0, sa1, sa2 = SA
    scale_p = [None] * 4
    scale_p[1] = np.asarray(inp["bn1_s"], np.float32) * sa1 / (SW * sa0)
    scale_p[2] = np.asarray(inp["bn2_s"], np.float32) * sa2 / (SW * sa1)
    scale_p[3] = np.asarray(inp["bn3_s"], np.float32) / (289.0 * SW * sa2)
    Cd = np.zeros((256, 3, 256), np.float32)
    for i, name in enumerate(("conv1_w", "conv2_w", "conv3_w")):
        w = np.asarray(inp[name], np.float32)                   # (256,256,5,5)
        wq8 = (w * SW).astype(ml_dtypes.float8_e4m3)
        dW = wq8.astype(np.float32) / SW - w                    # (cout,cin,5,5)
        Cl = np.einsum('ocyx,yx->co', dW, beta)                 # (cin, cout)
        Cd[:, i, :] = -(SW / 289.0) * Cl * scale_p[i + 1][None, :]
        t = wq8.transpose(1, 2, 3, 0)       # (cin, dy, dx, cout)
        t = t.reshape(2, 128, 5, 5, 2, 128)  # (kt, p, dy, dx, mt, m)
        t = t.transpose(1, 2, 3, 4, 0, 5)    # (p, dy, dx, mt, kt, m)
        g[f"w{i + 1}"] = np.ascontiguousarray(
            t.reshape(128, 25 * 2 * 2 * 128))
    t = Cd.reshape(2, 128, 3, 2, 128).transpose(1, 0, 2, 3, 4)
    g["corr"] = np.ascontiguousarray(t.reshape(128, 2 * 3 * 2 * 128))
    bn = np.zeros((256, 8), np.float32)
    folds = [(np.float32(sa0), np.float32(sa0)),
             (scale_p[1], np.float32(sa1)),
             (scale_p[2], np.float32(sa2)),
             (scale_p[3], np.float32(1.0 / 289.0))]
    for i in range(4):
        s = np.asarray(inp[f"bn{i}_s"], np.float32)
        b = np.asarray(inp[f"bn{i}_b"], np.float32)
        fs, fb = folds[i]
        bn[:, 2 * i] = s * fs if i == 0 else fs
        bn[:, 2 * i + 1] = b * fb
    g["bn"] = bn
    return g


_CACHE = {}


def kernel(**inputs):
    _install_ntff_hook()
    from concourse.bass_utils import run_bass_kernel_spmd

    if "nc" not in _CACHE:
        _CACHE["nc"] = build_program(NSAMP)
    nc = _CACHE["nc"]

    shared = prep_weights(inputs)
    xcat = make_xcat(inputs["x"])           # (1024, 17, 256)
    in_maps = []
    for c in range(NCORES):
        xc = xcat[c * NSAMP:(c + 1) * NSAMP]            # (128, 17, 256)
        xcT = np.ascontiguousarray(
            xc.transpose(2, 1, 0).reshape(256, L * NSAMP), dtype=np.float16)
        m = dict(shared)
        m["xcatT"] = xcT
        in_maps.append(m)

    res = run_bass_kernel_spmd(nc, in_maps, core_ids=list(range(NCORES)))
    out = np.zeros((B, DIM, T), np.float32)
    for c in range(NCORES):
        hc = res.results[c]["hout"].T                   # (128, 256)
        out[:, :, c::K] = hc.reshape(B, NW, DIM).transpose(0, 2, 1)
    return out

